# revision 23
# baseline (speedup 1.0000x reference)
"""Trainium2 Bass kernel: 5x5 reflect-padded box-filter mean (LocalMean).

Full input:  image (32, 3, 512, 512) f32
Full output: same shape; out[r,c] = mean of the 5x5 window of the
reflect-padded image.

Strategy (pure data parallel over 8 NeuronCores, 4 images per core;
HW is DMA-bandwidth bound at ~160 GB/s/core, so minimize bytes):
- Host reflect-pads, scales by 8 and converts to fp8 e3m4 (4 mantissa
  bits; subnormals only below 0.031), then lays the image out blocked so
  each image is ONE contiguous DMA: xt[n, p, c, b, w] =
  padded[n, c, 124*b + p, w].  The 4-row overlaps between the 124-row
  output blocks are duplicated host-side.
- On chip, per image: one [128, 6192] fp8 load; vertical 5-tap sum via
  banded matmul (weights exactly 1.0, PSUM accumulates f32) in
  1024-column PSUM chunks; ScalarE applies the 1/(25*8) mean scale while
  copying PSUM -> SBUF f16; horizontal 5-tap as a 3-op f16 add-tree
  fused across all 12 channel-blocks, block-split between DVE and Pool;
  level 3 writes a packed O tile through strided views so the f16 output
  DMA is fully contiguous.  The 20 tail rows of all 4 images are packed
  into one [80, 1548] tile and processed with a block-diag band matmul.
- Host reassembles the blocked f16 outputs and upcasts to f32.
  rel err ~1.0e-2 (fp8 input quantization), under the 2e-2 gate.
"""

import numpy as np

N_CORES = 8
B, C, H, W = 32, 3, 512, 512
PB = B // N_CORES          # images per core
PAD = 2
HP, WP = H + 2 * PAD, W + 2 * PAD   # 516

NB = 4                      # big row blocks per image (124 out rows each)
BH = 124                    # out rows per big block
BW = WP                     # block pitch in V space (516)
OW = W                      # valid out cols per block (512)
FREE = C * NB * BW          # 6192 free cols in the fused per-image tile
TR = 20                     # tail input rows per image (496..516)
TO = 16                     # tail output rows per image (496..512)
TFREE = C * BW              # 1548
CHUNK = 1024                # PSUM chunk (f32, exactly 2 banks)

_CACHE = {}
# pool_blocks: 516-col blocks of the horizontal add-tree handed to Pool
# (rest on DVE, split into dve_subranges so the tree starts before all
# PSUM->SBUF copies finish).  Pool adds run ~3.8x slower than DVE 2-byte
# adds.  Block-aligned so tree level 3 can write a packed output tile
# through a strided view (contiguous out-DMA, 12x fewer descriptors).
# in_dtype: "float8e3" (e3m4, x8 prescale, 1/200 folded into the ScalarE
# PSUM->SBUF copy) halves input DMA bytes vs "float16"; the f16 chain
# keeps rounding error ~1e-2 (fp8) / ~1e-3 (f16), under the 2e-2 gate.
_CFG = {"pool_blocks": 2, "dve_subranges": 2, "lookahead": 3,
        "in_dtype": "float8e3", "out_dma_eng": "sync"}


def _band(K, M, blocks=1):
    # W[k, m] = 1 for 0 <= k - m <= 4 within each diagonal block.
    kb, mb = K // blocks, M // blocks
    out = np.zeros((K, M), np.float32)
    for n in range(blocks):
        k = np.arange(kb)[:, None]
        m = np.arange(mb)[None, :]
        out[n * kb:(n + 1) * kb, n * mb:(n + 1) * mb] = (
            ((k - m) >= 0) & ((k - m) <= 4)).astype(np.float32)
    return out


def _build(reps=1):
    import concourse.bacc as bacc
    import concourse.tile as tile
    from concourse import mybir

    f32 = mybir.dt.float32
    f16 = mybir.dt.float16
    din = getattr(mybir.dt, _CFG["in_dtype"])
    nc = bacc.Bacc("TRN2", target_bir_lowering=False, debug=False,
                   num_devices=N_CORES)
    xt = nc.dram_tensor("xt", [PB, 128, C, NB, BW], din,
                        kind="ExternalInput").ap()
    xtl = nc.dram_tensor("xtl", [PB * TR, C, BW], din,
                         kind="ExternalInput").ap()
    wd = nc.dram_tensor("wd", [128, BH], din, kind="ExternalInput").ap()
    wt = nc.dram_tensor("wt", [PB * TR, PB * TO], din,
                        kind="ExternalInput").ap()
    yt = nc.dram_tensor("yt", [PB, BH, C, NB, OW], f16,
                        kind="ExternalOutput").ap()
    ytl = nc.dram_tensor("ytl", [PB * TO, C, OW], f16,
                         kind="ExternalOutput").ap()
    act_scale = (1.0 / 200.0 if _CFG["in_dtype"] == "float8e3" else None)

    LOOKAHEAD = _CFG["lookahead"]
    dve_blocks = C * NB - _CFG["pool_blocks"]  # DVE gets blocks [0, this)
    nsub = _CFG["dve_subranges"]

    with tile.TileContext(nc) as tc:
        with (
            tc.tile_pool(name="wp", bufs=1) as wp,
            tc.tile_pool(name="xp", bufs=LOOKAHEAD + 1) as xp,
            tc.tile_pool(name="xtp", bufs=2) as xtp,
            tc.tile_pool(name="vp", bufs=2, space="PSUM") as vp,
            tc.tile_pool(name="tp", bufs=2, space="PSUM") as tp,
            tc.tile_pool(name="vsp", bufs=2) as vsp,
            tc.tile_pool(name="s2p", bufs=6) as s2p,
            tc.tile_pool(name="s4p", bufs=6) as s4p,
            tc.tile_pool(name="op", bufs=2) as op,
        ):
            d_t = wp.tile([128, BH], din)
            nc.sync.dma_start(d_t[:], wd[:, :])
            t_t = wp.tile([PB * TR, PB * TO], din)
            nc.sync.dma_start(t_t[:], wt[:, :])

            steps = [s for _ in range(reps) for s in [0, 1, 2, 3, "tail"]]
            loaded = {}

            def load(si):
                s = steps[si]
                if s == "tail":
                    t = xtp.tile([PB * TR, TFREE], din)
                    nc.sync.dma_start(
                        t[:], xtl[:, :, :].rearrange("p c w -> p (c w)"))
                else:
                    t = xp.tile([128, FREE], din)
                    nc.sync.dma_start(
                        t[:], xt[s].rearrange("p c b w -> p (c b w)"))
                loaded[si] = t

            for si in range(min(LOOKAHEAD, len(steps))):
                load(si)

            for si, s in enumerate(steps):
                if si + LOOKAHEAD < len(steps):
                    load(si + LOOKAHEAD)
                X = loaded.pop(si)

                # ranges are (engine, b0, b1) in units of 516-col blocks
                if s == "tail":
                    ptn, w_t, free, nblk = PB * TO, t_t, TFREE, C
                    ranges = [("vector", 0, C)]
                else:
                    ptn, w_t, free, nblk = BH, d_t, FREE, C * NB
                    bounds = [dve_blocks * k // nsub for k in range(nsub)]
                    ranges = [("vector", b0, b1) for b0, b1 in
                              zip(bounds, bounds[1:] + [dve_blocks])]
                    if dve_blocks < nblk:
                        ranges.append(("gpsimd", dve_blocks, nblk))

                # Vertical 5-tap sum: banded matmul into f32 PSUM chunks
                # (1024 f32 = 2 banks), ScalarE copies PSUM -> SBUF bf16.
                # (+4 junk cols so block-aligned +4-offset views stay in
                # bounds; they are never read.)
                Vs = vsp.tile([ptn, free + 4], f16)
                nfull = free // CHUNK
                for k in range(nfull + (1 if free % CHUNK else 0)):
                    c0 = k * CHUNK
                    cw = min(CHUNK, free - c0)
                    pool = vp if (k < nfull and s != "tail") else tp
                    v = pool.tile([128, CHUNK], f32)
                    for m0 in range(0, cw, 512):
                        mw = min(512, cw - m0)
                        nc.tensor.matmul(v[0:ptn, m0:m0 + mw], w_t[:],
                                         X[:, c0 + m0:c0 + m0 + mw],
                                         start=True, stop=True)
                    if act_scale is not None:
                        nc.scalar.mul(Vs[:, c0:c0 + cw], v[0:ptn, 0:cw],
                                      act_scale)
                    else:
                        nc.scalar.copy(Vs[:, c0:c0 + cw], v[0:ptn, 0:cw])

                # Horizontal 5-tap: 3-op add tree, block-split across
                # engines with private temps (local coords); junk at the
                # 516-col block seams is never read back.  Level 3 writes
                # the packed O tile through strided per-block views so the
                # output DMA is fully contiguous.
                O = op.tile([ptn, nblk * OW], f16)
                for ename, b0, b1 in ranges:
                    eng = getattr(nc, ename)
                    c0, nb = b0 * BW, b1 - b0
                    L = min(nb * BW + 4, free - c0)
                    S2 = s2p.tile([ptn, L], f16)
                    eng.tensor_add(S2[:, 0:L - 1], Vs[:, c0:c0 + L - 1],
                                   Vs[:, c0 + 1:c0 + L])
                    S4 = s4p.tile([ptn, L], f16)
                    eng.tensor_add(S4[:, 0:L - 3], S2[:, 0:L - 3],
                                   S2[:, 2:L - 1])
                    s4v = S4[:, 0:nb * BW].rearrange(
                        "p (b w) -> p b w", w=BW)[:, :, 0:OW]
                    vsv = Vs[:, c0 + 4:c0 + 4 + nb * BW].rearrange(
                        "p (b w) -> p b w", w=BW)[:, :, 0:OW]
                    ov = O[:, b0 * OW:b1 * OW].rearrange(
                        "p (b w) -> p b w", w=OW)
                    eng.tensor_add(ov, s4v, vsv)

                # Contiguous output DMA, issued from SP *after* this
                # step's prefetch was issued.
                dst = (ytl[:, :, :].rearrange("p c w -> p (c w)")
                       if s == "tail" else
                       yt[s].rearrange("p c b w -> p (c b w)"))
                getattr(nc, _CFG["out_dma_eng"]).dma_start(dst, O[:])

    nc.compile()
    return nc


def _get_nc(reps=1):
    key = ("nc", reps)
    if key not in _CACHE:
        _CACHE[key] = _build(reps)
    return _CACHE[key]


def _shard_inputs(image: np.ndarray):
    import ml_dtypes

    if _CFG["in_dtype"] == "float8e3":
        dt_in, pre = ml_dtypes.float8_e3m4, 8.0
    else:
        dt_in, pre = np.float16, 1.0 / 25.0

    image = np.asarray(image, dtype=np.float32)
    padded = np.pad(image * np.float32(pre),
                    ((0, 0), (0, 0), (PAD, PAD), (PAD, PAD)),
                    mode="reflect")
    wd = _band(128, BH).astype(dt_in)
    wt = _band(PB * TR, PB * TO, blocks=PB).astype(dt_in)
    in_maps = []
    for i in range(N_CORES):
        p = padded[i * PB:(i + 1) * PB]            # [4, 3, 516, 516] f32
        # xt[n, p, c, b, w] = p[n, c, 124b + p, w]
        blocks = np.stack([p[:, :, BH * b:BH * b + 128, :]
                           for b in range(NB)], axis=2)  # [n, c, b, p, w]
        xt = np.ascontiguousarray(
            blocks.transpose(0, 3, 1, 2, 4)).astype(dt_in)
        # xtl[(n r), c, w] = p[n, c, 496 + r, w]
        xtl = np.ascontiguousarray(
            p[:, :, HP - TR:, :].transpose(0, 2, 1, 3)
        ).reshape(PB * TR, C, WP).astype(dt_in)
        in_maps.append({"xt": xt, "xtl": xtl, "wd": wd, "wt": wt})
    return in_maps


def kernel(image: np.ndarray) -> np.ndarray:
    from concourse import bass_utils

    nc = _get_nc()
    in_maps = _shard_inputs(image)
    res = bass_utils.run_bass_kernel_spmd(nc, in_maps,
                                          core_ids=list(range(N_CORES)))
    out = np.empty((B, C, H, W), np.float32)
    for i in range(N_CORES):
        yt = np.asarray(res.results[i]["yt"], dtype=np.float32)
        ytl = np.asarray(res.results[i]["ytl"], dtype=np.float32)
        # yt[n, p, c, b, w] -> rows 124b + p
        big = yt.transpose(0, 2, 3, 1, 4).reshape(PB, C, NB * BH, W)
        out[i * PB:(i + 1) * PB, :, 0:NB * BH, :] = big
        tl = ytl.reshape(PB, TO, C, W).transpose(0, 2, 1, 3)
        out[i * PB:(i + 1) * PB, :, NB * BH:, :] = tl
    return out
